# revision 15
# baseline (speedup 1.0000x reference)
"""Trainium2 Bass kernel for nn_DiffNNLS: 2-layer MLP transform + 100-iter projected-gradient NNLS.

Strategy: pure data-parallel over batch (4096 -> 8 cores x 512). Per core:
  P0: corrected = ref_profiles + sigmoid(profile_correction); corrT via PE transpose
  P1: hT = relu(W1 @ mpT + b1)              (marker-major, fp32r matmuls)
  P2: wT = (W2 @ hT + b2) * covT            (kept in SBUF)
  P3: per-sample Gram matrices A_b = corr diag(cov_b) corr^T via one matmul
      cov^T-contract against pairwise-product columns Q; G = wT^T-contract corrT.
      A and -G packed into augmented tensor Aaug[b, i, j(33)].
  P4: NNLS iterations on DVE: grad-G = reduce_j(Aaug * x_aug); x = normalize(relu(x - lr*(...)))
      With period-2-constant step sizes, x4 == x2 bitwise implies a 2-cycle, so
      iterations 4..N-1 are branch-skipped when the fixed cycle is detected.
  P5: reconstruction = (x @ ref_profiles) * cov
"""
import numpy as np
from contextlib import ExitStack

import concourse.bass as bass
import concourse.tile as tile
from concourse import bacc, mybir
from concourse.bass_utils import run_bass_kernel_spmd

F32 = mybir.dt.float32
F32R = mybir.dt.float32r
AX = mybir.AxisListType
OP = mybir.AluOpType
ACTF = mybir.ActivationFunctionType

B, M, C = 4096, 5000, 32
N_CORES = 8
BL = B // N_CORES            # 512 samples per core
NBC = BL // 128              # 4 batch chunks of 128
MP = 5120                    # padded marker dim (40 * 128)
MT = MP // 128               # 40 marker tiles
KSLABS = MP // 1024          # 5 slabs of 1024 output features


def build_program(lrs, n_iters, phases=6, force_fast_path=False):
    """Emit the SPMD program. lrs: list of per-iteration step sizes (floats)."""
    nc = bacc.Bacc("TRN2", target_bir_lowering=False, debug=False, num_devices=N_CORES)

    mpT_t = nc.dram_tensor("mpT", [MP, BL], F32R, kind="ExternalInput")
    covT_t = nc.dram_tensor("covT", [MP, BL], F32R, kind="ExternalInput")
    covB_t = nc.dram_tensor("covB", [BL, MP], F32, kind="ExternalInput")
    w1T_t = nc.dram_tensor("W1T", [MP, MP], F32R, kind="ExternalInput")
    w2T_t = nc.dram_tensor("W2T", [MP, MP], F32R, kind="ExternalInput")
    b1t_t = nc.dram_tensor("b1t", [128, MT], F32, kind="ExternalInput")
    b2t_t = nc.dram_tensor("b2t", [128, MT], F32, kind="ExternalInput")
    pc_t = nc.dram_tensor("pc", [C, MP], F32, kind="ExternalInput")
    rp_t = nc.dram_tensor("rp", [C, MP], F32R, kind="ExternalInput")
    id_t = nc.dram_tensor("ident", [128, 128], F32R, kind="ExternalInput")
    xout_t = nc.dram_tensor("xout", [BL, C], F32, kind="ExternalOutput")
    recon_t = nc.dram_tensor("recon", [BL, MP], F32, kind="ExternalOutput")

    # early-exit is sound when the iteration map repeats with period 2 from iter 2 on
    n_uncond = 4 if n_iters % 2 == 0 else 5
    cycle_ok = n_iters > n_uncond and all(
        lrs[t] == lrs[t - 2] for t in range(n_uncond - 2, n_iters))

    with tile.TileContext(nc) as tc:
        with ExitStack() as top:
            # ---- long-lived tiles ----
            persist = top.enter_context(tc.tile_pool(name="persist", bufs=1))
            corrT_sb = persist.tile([128, MT, C], F32R)       # 5.1KB/part
            b1t_sb = persist.tile([128, MT], F32)
            b2t_sb = persist.tile([128, MT], F32)
            ident_sb = persist.tile([128, 128], F32R)
            nc.scalar.dma_start(b1t_sb[:], b1t_t.ap())
            nc.scalar.dma_start(b2t_sb[:], b2t_t.ap())
            nc.scalar.dma_start(ident_sb[:], id_t.ap())

            aaug_sb = top.enter_context(tc.tile_pool(name="aaug", bufs=1)).tile(
                [128, NBC, C, C + 1], F32, name="aaug_sb")
            loop_pool = top.enter_context(tc.tile_pool(name="loop", bufs=1))
            xaug = loop_pool.tile([128, NBC, C + 1], F32, name="xaug")

            if phases < 0:
                stub = loop_pool.tile([128, C], F32, name="stub")
                nc.vector.memset(stub[:], 0.0)
                nc.sync.dma_start(xout_t.ap().rearrange("(c p) j -> p c j", p=128)[:, 0, :],
                                  stub[:])

            # ================= P0: corrected profiles, transposed =================
            if phases >= 0:
                with ExitStack() as ctx:
                    pool = ctx.enter_context(tc.tile_pool(name="p0", bufs=1))
                    ps = ctx.enter_context(tc.tile_pool(name="p0ps", bufs=4, space="PSUM"))
                    pc_sb = pool.tile([C, MP], F32)
                    nc.scalar.dma_start(pc_sb[:], pc_t.ap())
                    rp_sb0 = pool.tile([C, MP], F32R)
                    nc.scalar.dma_start(rp_sb0[:], rp_t.ap())
                    sig_sb = pool.tile([C, MP], F32)
                    nc.scalar.activation(sig_sb[:], pc_sb[:], ACTF.Sigmoid)
                    corr_sb = pool.tile([C, MP], F32R)
                    nc.vector.tensor_tensor(corr_sb[:], rp_sb0[:].bitcast(F32),
                                            sig_sb[:], op=OP.add)
                    for mt in range(MT):
                        ptr = ps.tile([128, C], F32R, tag="ptr", name="ptr")
                        nc.tensor.transpose(ptr[:], corr_sb[:, 128 * mt:128 * (mt + 1)],
                                            ident_sb[0:C, 0:C])
                        nc.scalar.copy(corrT_sb[:, mt, :], ptr[:])

            # ================= P1: hT = relu(W1 @ mpT + b1) =================
            big_stack = ExitStack()
            big_pool = big_stack.enter_context(tc.tile_pool(name="big", bufs=2))
            mpT_sb = big_pool.tile([128, MT, BL], F32R, tag="big", name="mpT_sb")
            hT_sb = big_pool.tile([128, MT, BL], F32R, tag="big", name="hT_sb")
            if phases >= 1:
                with ExitStack() as ctx:
                    w_pool = ctx.enter_context(tc.tile_pool(name="w1s", bufs=5))
                    ps = ctx.enter_context(tc.tile_pool(name="p1ps", bufs=8, space="PSUM"))
                    nc.scalar.dma_start(mpT_sb[:],
                                        mpT_t.ap().rearrange("(t p) c -> p t c", p=128))
                    for s in range(KSLABS):
                        accs = [ps.tile([128, BL], F32, tag="acc", name="acc")
                                for _ in range(8)]
                        for mt in range(MT):
                            w1t = w_pool.tile([128, 1024], F32R, tag="w", name="w1t")
                            nc.sync.dma_start(
                                w1t[:], w1T_t.ap()[128 * mt:128 * (mt + 1),
                                                   1024 * s:1024 * (s + 1)])
                            for jj in range(8):
                                nc.tensor.matmul(accs[jj][:],
                                                 w1t[:, 128 * jj:128 * (jj + 1)],
                                                 mpT_sb[:, mt, :],
                                                 start=(mt == 0), stop=(mt == MT - 1))
                        for jj in range(8):
                            kt = 8 * s + jj
                            if jj < 4:
                                nc.scalar.activation(hT_sb[:, kt, :], accs[jj][:],
                                                     ACTF.Relu,
                                                     bias=b1t_sb[:, kt:kt + 1])
                            else:
                                nc.vector.tensor_scalar(hT_sb[:, kt, :], accs[jj][:],
                                                        b1t_sb[:, kt:kt + 1], 0.0,
                                                        op0=OP.add, op1=OP.max)

            # ================= P2: wT = (W2 @ hT + b2) * covT (SBUF-resident) ========
            wT_sb = big_pool.tile([128, MT, BL], F32R, tag="big", name="wT_sb")
            if phases >= 2:
                with ExitStack() as ctx:
                    w_pool = ctx.enter_context(tc.tile_pool(name="w2s", bufs=4))
                    cv_pool = ctx.enter_context(tc.tile_pool(name="cv", bufs=3))
                    ps = ctx.enter_context(tc.tile_pool(name="p2ps", bufs=8, space="PSUM"))
                    for s in range(KSLABS):
                        accs = [ps.tile([128, BL], F32, tag="acc", name="acc")
                                for _ in range(8)]
                        for kt in range(MT):
                            w2t = w_pool.tile([128, 1024], F32R, tag="w", name="w2t")
                            nc.sync.dma_start(
                                w2t[:], w2T_t.ap()[128 * kt:128 * (kt + 1),
                                                   1024 * s:1024 * (s + 1)])
                            for jj in range(8):
                                nc.tensor.matmul(accs[jj][:],
                                                 w2t[:, 128 * jj:128 * (jj + 1)],
                                                 hT_sb[:, kt, :],
                                                 start=(kt == 0), stop=(kt == MT - 1))
                        for jj in range(8):
                            mt2 = 8 * s + jj
                            cvt = cv_pool.tile([128, BL], F32R, tag="cv", name="cvt")
                            nc.scalar.dma_start(cvt[:],
                                                covT_t.ap()[128 * mt2:128 * (mt2 + 1), :])
                            nc.vector.scalar_tensor_tensor(
                                wT_sb[:, mt2, :], accs[jj][:], b2t_sb[:, mt2:mt2 + 1],
                                cvt[:].bitcast(F32), op0=OP.add, op1=OP.mult)

            # ================= P3: Aaug = [A | -G] =================
            if phases >= 3:
                # G pass: G[b,i] = sum_m wT[m,b] corrT[m,i]
                with ExitStack() as ctx:
                    ps = ctx.enter_context(tc.tile_pool(name="p3g", bufs=1, space="PSUM"))
                    pG = [ps.tile([128, C], F32, tag=f"pg{c}", name=f"pg{c}")
                          for c in range(NBC)]
                    for mt in range(MT):
                        st, sp = (mt == 0), (mt == MT - 1)
                        for c in range(NBC):
                            nc.tensor.matmul(pG[c][:],
                                             wT_sb[:, mt, 128 * c:128 * (c + 1)],
                                             corrT_sb[:, mt, :], start=st, stop=sp)
                    for c in range(NBC):
                        nc.vector.tensor_scalar(aaug_sb[:, c, :, C], pG[c][:], -1.0,
                                                None, op0=OP.mult)
                # A pass: Avec[b,(i,j)] = sum_m covT[m,b] corr[i,m] corr[j,m]
                with ExitStack() as ctx:
                    cv_pool = ctx.enter_context(tc.tile_pool(name="cv3", bufs=3))
                    q_pool = ctx.enter_context(tc.tile_pool(name="q3", bufs=3))
                    ps = ctx.enter_context(tc.tile_pool(name="p3a", bufs=1, space="PSUM"))
                    pA = [[ps.tile([128, 512], F32, tag=f"pa{c}{h}", name=f"pa{c}{h}")
                           for h in range(2)] for c in range(NBC)]
                    for mt in range(MT):
                        cvt = cv_pool.tile([128, BL], F32R, tag="cv", name="cvt3")
                        nc.scalar.dma_start(cvt[:],
                                            covT_t.ap()[128 * mt:128 * (mt + 1), :])
                        qt = q_pool.tile([128, C * C], F32R, tag="q", name="qt")
                        ct = corrT_sb[:, mt, :]
                        nc.vector.tensor_tensor(
                            qt[:].rearrange("p (i j) -> p i j", i=C),
                            ct.unsqueeze(2).broadcast_to([128, C, C]),
                            ct.unsqueeze(1).broadcast_to([128, C, C]), op=OP.mult)
                        st, sp = (mt == 0), (mt == MT - 1)
                        for c in range(NBC):
                            lhs_cv = cvt[:, 128 * c:128 * (c + 1)]
                            nc.tensor.matmul(pA[c][0][:], lhs_cv, qt[:, 0:512],
                                             start=st, stop=sp)
                            nc.tensor.matmul(pA[c][1][:], lhs_cv, qt[:, 512:1024],
                                             start=st, stop=sp)
                    for c in range(NBC):
                        for h in range(2):
                            nc.vector.tensor_copy(
                                aaug_sb[:, c, 16 * h:16 * (h + 1), 0:C],
                                pA[c][h][:].rearrange("p (i j) -> p i j", i=16))

            big_stack.close()

            # ================= P4: the NNLS loop =================
            if phases >= 4:
                nc.vector.memset(xaug[:, :, 0:C], 0.0)
                nc.vector.memset(xaug[:, :, C:C + 1], 1.0)
                with ExitStack() as ctx:
                    it_pool = ctx.enter_context(tc.tile_pool(name="it", bufs=1))

                    def emit_iter(t):
                        lr = float(lrs[t])
                        tt = it_pool.tile([128, NBC, C, C + 1], F32, tag="tt", name="tt")
                        nc.vector.tensor_tensor(
                            tt[:], aaug_sb[:],
                            xaug[:].unsqueeze(2).broadcast_to([128, NBC, C, C + 1]),
                            op=OP.mult)
                        grad = it_pool.tile([128, NBC, C], F32, tag="grad", name="grad")
                        nc.vector.tensor_reduce(grad[:], tt[:], axis=AX.X, op=OP.add)
                        u = it_pool.tile([128, NBC, C], F32, tag="u", name="u")
                        nc.vector.scalar_tensor_tensor(u[:], grad[:], -lr,
                                                       xaug[:, :, 0:C],
                                                       op0=OP.mult, op1=OP.add)
                        v = it_pool.tile([128, NBC, C], F32, tag="v", name="v")
                        nc.vector.tensor_scalar(v[:], u[:], 0.0, None, op0=OP.max)
                        s = it_pool.tile([128, NBC, 1], F32, tag="s", name="s")
                        nc.vector.tensor_reduce(s[:], v[:], axis=AX.X, op=OP.add)
                        se = it_pool.tile([128, NBC], F32, tag="se", name="se")
                        nc.vector.tensor_scalar(se[:], s[:].squeeze(2), 1e-8, None,
                                                op0=OP.add)
                        rcp = it_pool.tile([128, NBC], F32, tag="rcp", name="rcp")
                        nc.vector.reciprocal(rcp[:], se[:])
                        nc.vector.tensor_tensor(
                            xaug[:, :, 0:C], v[:],
                            rcp[:].unsqueeze(2).broadcast_to([128, NBC, C]), op=OP.mult)

                    if cycle_ok:
                        xsnap = loop_pool.tile([128, NBC, C], F32)
                        flag_sb = loop_pool.tile([1, 1], F32)
                        ones_sb = loop_pool.tile([128, 1], F32)
                        nc.vector.memset(ones_sb[:], 1.0)
                        for t in range(n_uncond - 2):
                            emit_iter(t)
                        nc.vector.tensor_copy(xsnap[:], xaug[:, :, 0:C])
                        emit_iter(n_uncond - 2)
                        emit_iter(n_uncond - 1)
                        diff = it_pool.tile([128, NBC, C], F32, tag="u", name="diff")
                        nc.vector.tensor_tensor(diff[:], xaug[:, :, 0:C], xsnap[:],
                                                op=OP.subtract)
                        dmax = loop_pool.tile([128, 1], F32)
                        nc.vector.tensor_reduce(dmax[:],
                                                diff[:].rearrange("p c j -> p (c j)"),
                                                axis=AX.X, op=OP.max,
                                                apply_absolute_value=True)
                        with tc.tile_pool(name="flagps", bufs=1, space="PSUM") as fps:
                            fpsum = fps.tile([1, 1], F32)
                            nc.tensor.matmul(fpsum[:], ones_sb[:], dmax[:],
                                             start=True, stop=True)
                            nc.vector.tensor_copy(flag_sb[:], fpsum[:])
                        flag_reg = nc.vector.alloc_register("flag_reg")
                        nc.vector.reg_load(flag_reg,
                                           flag_sb[0:1, 0:1].bitcast(mybir.dt.uint32))
                        rv = nc.vector.snap(flag_reg)
                        if not force_fast_path:
                            with tc.If(rv >= 1):
                                for t in range(n_uncond, n_iters):
                                    emit_iter(t)
                    else:
                        for t in range(n_iters):
                            emit_iter(t)

                # write x out
                nc.sync.dma_start(
                    xout_t.ap().rearrange("(c p) j -> p c j", p=128), xaug[:, :, 0:C])

            # ================= P5: recon = (x @ rp) * cov =================
            if phases >= 5:
                with ExitStack() as ctx:
                    pool = ctx.enter_context(tc.tile_pool(name="p5", bufs=1))
                    io_pool = ctx.enter_context(tc.tile_pool(name="p5io", bufs=4))
                    ps = ctx.enter_context(tc.tile_pool(name="p5ps", bufs=1, space="PSUM"))
                    rp_sb = pool.tile([C, MP], F32R)
                    nc.scalar.dma_start(rp_sb[:], rp_t.ap())
                    xT_sb = pool.tile([C, NBC, 128], F32R)
                    for c in range(NBC):
                        ptx = ps.tile([C, 128], F32, tag="ptx", name="ptx", bufs=2)
                        nc.tensor.transpose(ptx[:], xaug[:, c, 0:C],
                                            ident_sb[:].bitcast(F32))
                        nc.scalar.copy(xT_sb[:, c, :], ptx[:])
                    for c in range(NBC):
                        for ns in range(MT // 4):
                            pr = ps.tile([128, 512], F32, tag="pr", name="pr", bufs=4)
                            nc.tensor.matmul(pr[:], xT_sb[:, c, :],
                                             rp_sb[:, 512 * ns:512 * (ns + 1)],
                                             start=True, stop=True)
                            cvb = io_pool.tile([128, 512], F32, tag="cvb", name="cvb")
                            nc.scalar.dma_start(
                                cvb[:], covB_t.ap()[128 * c:128 * (c + 1),
                                                    512 * ns:512 * (ns + 1)])
                            rt = io_pool.tile([128, 512], F32, tag="rt", name="rt")
                            nc.vector.tensor_tensor(rt[:], pr[:], cvb[:], op=OP.mult)
                            nc.sync.dma_start(
                                recon_t.ap()[128 * c:128 * (c + 1),
                                             512 * ns:512 * (ns + 1)], rt[:])

    nc.compile()
    return nc


_CACHE = {}


def _get_program(step_sizes):
    key = step_sizes.astype(np.float32).tobytes()
    if _CACHE.get("key") != key:
        _CACHE["nc"] = build_program([float(v) for v in step_sizes], len(step_sizes))
        _CACHE["key"] = key
    return _CACHE["nc"]


def _pad_cols(a, width):
    out = np.zeros((a.shape[0], width), dtype=np.float32)
    out[:, :a.shape[1]] = a
    return out


def kernel(marker_props, coverage, W1, b1, W2, b2, step_sizes,
           profile_correction, reference_profiles):
    marker_props = np.asarray(marker_props, dtype=np.float32)
    coverage = np.asarray(coverage, dtype=np.float32)
    W1 = np.asarray(W1, dtype=np.float32)
    b1 = np.asarray(b1, dtype=np.float32)
    W2 = np.asarray(W2, dtype=np.float32)
    b2 = np.asarray(b2, dtype=np.float32)
    step_sizes = np.asarray(step_sizes, dtype=np.float32)
    profile_correction = np.asarray(profile_correction, dtype=np.float32)
    reference_profiles = np.asarray(reference_profiles, dtype=np.float32)

    nc = _get_program(step_sizes)

    # host-side layout prep (pads + transposes)
    w1T = np.zeros((MP, MP), dtype=np.float32)
    w1T[:M, :M] = W1.T
    w2T = np.zeros((MP, MP), dtype=np.float32)
    w2T[:M, :M] = W2.T
    mpT = np.zeros((MP, B), dtype=np.float32)
    mpT[:M] = marker_props.T
    covT = np.zeros((MP, B), dtype=np.float32)
    covT[:M] = coverage.T
    covB = _pad_cols(coverage, MP)
    b1t = np.zeros(MP, dtype=np.float32)
    b1t[:M] = b1
    b1t = np.ascontiguousarray(b1t.reshape(MT, 128).T)
    b2t = np.zeros(MP, dtype=np.float32)
    b2t[:M] = b2
    b2t = np.ascontiguousarray(b2t.reshape(MT, 128).T)
    pc = _pad_cols(profile_correction, MP)
    rp = _pad_cols(reference_profiles, MP)
    ident = np.eye(128, dtype=np.float32)

    in_maps = []
    for c in range(N_CORES):
        sl = slice(BL * c, BL * (c + 1))
        in_maps.append({
            "mpT": np.ascontiguousarray(mpT[:, sl]),
            "covT": np.ascontiguousarray(covT[:, sl]),
            "covB": np.ascontiguousarray(covB[sl]),
            "W1T": w1T, "W2T": w2T,
            "b1t": b1t, "b2t": b2t,
            "pc": pc, "rp": rp, "ident": ident,
        })

    res = run_bass_kernel_spmd(nc, in_maps, core_ids=list(range(N_CORES)))
    x = np.concatenate([res.results[c]["xout"] for c in range(N_CORES)], axis=0)
    recon = np.concatenate([res.results[c]["recon"][:, :M] for c in range(N_CORES)],
                           axis=0)
    return x, np.zeros_like(x), recon


# revision 17
# speedup vs baseline: 1.0104x; 1.0104x over previous
"""Trainium2 Bass kernel for nn_DiffNNLS: 2-layer MLP transform + 100-iter projected-gradient NNLS.

Strategy: pure data-parallel over batch (4096 -> 8 cores x 512). Per core:
  P0: corrected = ref_profiles + sigmoid(profile_correction); corrT via PE transpose
  P1: hT = relu(W1 @ mpT + b1)              (marker-major, fp32r matmuls)
  P2: wT = (W2 @ hT + b2) * covT            (kept in SBUF)
  P3: per-sample Gram matrices A_b = corr diag(cov_b) corr^T via one matmul
      cov^T-contract against pairwise-product columns Q; G = wT^T-contract corrT.
      A and -G packed into augmented tensor Aaug[b, i, j(33)].
  P4: NNLS iterations on DVE: grad-G = reduce_j(Aaug * x_aug); x = normalize(relu(x - lr*(...)))
      With period-2-constant step sizes, x4 == x2 bitwise implies a 2-cycle, so
      iterations 4..N-1 are branch-skipped when the fixed cycle is detected.
  P5: reconstruction = (x @ ref_profiles) * cov
"""
import numpy as np
from contextlib import ExitStack

import concourse.bass as bass
import concourse.tile as tile
from concourse import bacc, mybir
from concourse.bass_utils import run_bass_kernel_spmd

F32 = mybir.dt.float32
F32R = mybir.dt.float32r
AX = mybir.AxisListType
OP = mybir.AluOpType
ACTF = mybir.ActivationFunctionType

B, M, C = 4096, 5000, 32
N_CORES = 8
BL = B // N_CORES            # 512 samples per core
NBC = BL // 128              # 4 batch chunks of 128
MP = 5120                    # padded marker dim (40 * 128)
MT = MP // 128               # 40 marker tiles
KSLABS = MP // 1024          # 5 slabs of 1024 output features


def build_program(lrs, n_iters, phases=6, force_fast_path=False):
    """Emit the SPMD program. lrs: list of per-iteration step sizes (floats)."""
    nc = bacc.Bacc("TRN2", target_bir_lowering=False, debug=False, num_devices=N_CORES)

    mpT_t = nc.dram_tensor("mpT", [MP, BL], F32R, kind="ExternalInput")
    covT_t = nc.dram_tensor("covT", [MP, BL], F32R, kind="ExternalInput")
    covB_t = nc.dram_tensor("covB", [BL, MP], F32, kind="ExternalInput")
    w1T_t = nc.dram_tensor("W1T", [MP, MP], F32R, kind="ExternalInput")
    w2T_t = nc.dram_tensor("W2T", [MP, MP], F32R, kind="ExternalInput")
    b1t_t = nc.dram_tensor("b1t", [128, MT], F32, kind="ExternalInput")
    b2t_t = nc.dram_tensor("b2t", [128, MT], F32, kind="ExternalInput")
    pc_t = nc.dram_tensor("pc", [C, MP], F32, kind="ExternalInput")
    rp_t = nc.dram_tensor("rp", [C, MP], F32R, kind="ExternalInput")
    id_t = nc.dram_tensor("ident", [128, 128], F32R, kind="ExternalInput")
    xout_t = nc.dram_tensor("xout", [BL, C], F32, kind="ExternalOutput")
    recon_t = nc.dram_tensor("recon", [BL, MP], F32, kind="ExternalOutput")

    # early-exit is sound when the iteration map repeats with period 2 from iter 2 on
    n_uncond = 4 if n_iters % 2 == 0 else 5
    cycle_ok = n_iters > n_uncond and all(
        lrs[t] == lrs[t - 2] for t in range(n_uncond - 2, n_iters))

    with tile.TileContext(nc) as tc:
        with ExitStack() as top:
            # ---- long-lived tiles ----
            persist = top.enter_context(tc.tile_pool(name="persist", bufs=1))
            corrT_sb = persist.tile([128, MT, C], F32R)       # 5.1KB/part
            b1t_sb = persist.tile([128, MT], F32)
            b2t_sb = persist.tile([128, MT], F32)
            ident_sb = persist.tile([128, 128], F32R)
            nc.scalar.dma_start(b1t_sb[:], b1t_t.ap())
            nc.scalar.dma_start(b2t_sb[:], b2t_t.ap())
            nc.scalar.dma_start(ident_sb[:], id_t.ap())

            aaug_sb = top.enter_context(tc.tile_pool(name="aaug", bufs=1)).tile(
                [128, NBC, C, C + 1], F32, name="aaug_sb")
            loop_pool = top.enter_context(tc.tile_pool(name="loop", bufs=1))
            xaug = loop_pool.tile([128, NBC, C + 1], F32, name="xaug")

            if phases < 0:
                stub = loop_pool.tile([128, C], F32, name="stub")
                nc.vector.memset(stub[:], 0.0)
                nc.sync.dma_start(xout_t.ap().rearrange("(c p) j -> p c j", p=128)[:, 0, :],
                                  stub[:])

            # ================= P0: corrected profiles, transposed =================
            if phases >= 0:
                with ExitStack() as ctx:
                    pool = ctx.enter_context(tc.tile_pool(name="p0", bufs=1))
                    ps = ctx.enter_context(tc.tile_pool(name="p0ps", bufs=4, space="PSUM"))
                    pc_sb = pool.tile([C, MP], F32)
                    nc.scalar.dma_start(pc_sb[:], pc_t.ap())
                    rp_sb0 = pool.tile([C, MP], F32R)
                    nc.scalar.dma_start(rp_sb0[:], rp_t.ap())
                    sig_sb = pool.tile([C, MP], F32)
                    nc.scalar.activation(sig_sb[:], pc_sb[:], ACTF.Sigmoid)
                    corr_sb = pool.tile([C, MP], F32R)
                    nc.vector.tensor_tensor(corr_sb[:], rp_sb0[:].bitcast(F32),
                                            sig_sb[:], op=OP.add)
                    for mt in range(MT):
                        ptr = ps.tile([128, C], F32R, tag="ptr", name="ptr")
                        nc.tensor.transpose(ptr[:], corr_sb[:, 128 * mt:128 * (mt + 1)],
                                            ident_sb[0:C, 0:C])
                        nc.scalar.copy(corrT_sb[:, mt, :], ptr[:])

            # ================= P1: hT = relu(W1 @ mpT + b1) =================
            big_stack = ExitStack()
            big_pool = big_stack.enter_context(tc.tile_pool(name="big", bufs=2))
            mpT_sb = big_pool.tile([128, MT, BL], F32R, tag="big", name="mpT_sb")
            hT_sb = big_pool.tile([128, MT, BL], F32R, tag="big", name="hT_sb")
            if phases >= 1:
                with ExitStack() as ctx:
                    w_pool = ctx.enter_context(tc.tile_pool(name="w1s", bufs=5))
                    ps = ctx.enter_context(tc.tile_pool(name="p1ps", bufs=8, space="PSUM"))
                    nc.scalar.dma_start(mpT_sb[:],
                                        mpT_t.ap().rearrange("(t p) c -> p t c", p=128))
                    for s in range(KSLABS):
                        accs = [ps.tile([128, BL], F32, tag="acc", name="acc")
                                for _ in range(8)]
                        for mt in range(MT):
                            w1t = w_pool.tile([128, 1024], F32R, tag="w", name="w1t")
                            nc.sync.dma_start(
                                w1t[:], w1T_t.ap()[128 * mt:128 * (mt + 1),
                                                   1024 * s:1024 * (s + 1)])
                            for jj in range(8):
                                nc.tensor.matmul(accs[jj][:],
                                                 w1t[:, 128 * jj:128 * (jj + 1)],
                                                 mpT_sb[:, mt, :],
                                                 start=(mt == 0), stop=(mt == MT - 1))
                        for jj in range(8):
                            kt = 8 * s + jj
                            if jj < 4:
                                nc.scalar.activation(hT_sb[:, kt, :], accs[jj][:],
                                                     ACTF.Relu,
                                                     bias=b1t_sb[:, kt:kt + 1])
                            else:
                                nc.vector.tensor_scalar(hT_sb[:, kt, :], accs[jj][:],
                                                        b1t_sb[:, kt:kt + 1], 0.0,
                                                        op0=OP.add, op1=OP.max)

            # ================= P2: wT = (W2 @ hT + b2) * covT (SBUF-resident) ========
            wT_sb = big_pool.tile([128, MT, BL], F32R, tag="big", name="wT_sb")
            if phases >= 2:
                with ExitStack() as ctx:
                    w_pool = ctx.enter_context(tc.tile_pool(name="w2s", bufs=4))
                    cv_pool = ctx.enter_context(tc.tile_pool(name="cv", bufs=3))
                    ps = ctx.enter_context(tc.tile_pool(name="p2ps", bufs=8, space="PSUM"))
                    for s in range(KSLABS):
                        accs = [ps.tile([128, BL], F32, tag="acc", name="acc")
                                for _ in range(8)]
                        for kt in range(MT):
                            w2t = w_pool.tile([128, 1024], F32R, tag="w", name="w2t")
                            nc.sync.dma_start(
                                w2t[:], w2T_t.ap()[128 * kt:128 * (kt + 1),
                                                   1024 * s:1024 * (s + 1)])
                            for jj in range(8):
                                nc.tensor.matmul(accs[jj][:],
                                                 w2t[:, 128 * jj:128 * (jj + 1)],
                                                 hT_sb[:, kt, :],
                                                 start=(kt == 0), stop=(kt == MT - 1))
                        for jj in range(8):
                            mt2 = 8 * s + jj
                            cvt = cv_pool.tile([128, BL], F32R, tag="cv", name="cvt")
                            nc.scalar.dma_start(cvt[:],
                                                covT_t.ap()[128 * mt2:128 * (mt2 + 1), :])
                            nc.vector.scalar_tensor_tensor(
                                wT_sb[:, mt2, :], accs[jj][:], b2t_sb[:, mt2:mt2 + 1],
                                cvt[:].bitcast(F32), op0=OP.add, op1=OP.mult)

            # ================= P3: Aaug = [A | -G] =================
            if phases >= 3:
                # G pass: G[b,i] = sum_m wT[m,b] corrT[m,i]
                with ExitStack() as ctx:
                    ps = ctx.enter_context(tc.tile_pool(name="p3g", bufs=1, space="PSUM"))
                    pG = [ps.tile([128, C], F32, tag=f"pg{c}", name=f"pg{c}")
                          for c in range(NBC)]
                    for mt in range(MT):
                        st, sp = (mt == 0), (mt == MT - 1)
                        for c in range(NBC):
                            nc.tensor.matmul(pG[c][:],
                                             wT_sb[:, mt, 128 * c:128 * (c + 1)],
                                             corrT_sb[:, mt, :], start=st, stop=sp)
                    for c in range(NBC):
                        nc.vector.tensor_scalar(aaug_sb[:, c, :, C], pG[c][:], -1.0,
                                                None, op0=OP.mult)
                # A pass: Avec[b,(i,j)] = sum_m covT[m,b] corr[i,m] corr[j,m]
                with ExitStack() as ctx:
                    cv_pool = ctx.enter_context(tc.tile_pool(name="cv3", bufs=3))
                    q_pool = ctx.enter_context(tc.tile_pool(name="q3", bufs=3))
                    ps = ctx.enter_context(tc.tile_pool(name="p3a", bufs=1, space="PSUM"))
                    pA = [[ps.tile([128, 512], F32, tag=f"pa{c}{h}", name=f"pa{c}{h}")
                           for h in range(2)] for c in range(NBC)]
                    for mt in range(MT):
                        cvt = cv_pool.tile([128, BL], F32R, tag="cv", name="cvt3")
                        nc.scalar.dma_start(cvt[:],
                                            covT_t.ap()[128 * mt:128 * (mt + 1), :])
                        qt = q_pool.tile([128, C * C], F32R, tag="q", name="qt")
                        ct = corrT_sb[:, mt, :]
                        nc.vector.tensor_tensor(
                            qt[:].rearrange("p (i j) -> p i j", i=C),
                            ct.unsqueeze(2).broadcast_to([128, C, C]),
                            ct.unsqueeze(1).broadcast_to([128, C, C]), op=OP.mult)
                        st, sp = (mt == 0), (mt == MT - 1)
                        for c in range(NBC):
                            lhs_cv = cvt[:, 128 * c:128 * (c + 1)]
                            nc.tensor.matmul(pA[c][0][:], lhs_cv, qt[:, 0:512],
                                             start=st, stop=sp)
                            nc.tensor.matmul(pA[c][1][:], lhs_cv, qt[:, 512:1024],
                                             start=st, stop=sp)
                    for c in range(NBC):
                        for h in range(2):
                            nc.vector.tensor_copy(
                                aaug_sb[:, c, 16 * h:16 * (h + 1), 0:C],
                                pA[c][h][:].rearrange("p (i j) -> p i j", i=16))

            big_stack.close()

            # ================= P4: the NNLS loop =================
            if phases >= 4:
                nc.vector.memset(xaug[:, :, 0:C], 0.0)
                nc.vector.memset(xaug[:, :, C:C + 1], 1.0)
                with ExitStack() as ctx:
                    it_pool = ctx.enter_context(tc.tile_pool(name="it", bufs=1))

                    def emit_iter(t):
                        lr = float(lrs[t])
                        u = it_pool.tile([128, NBC, C], F32, tag="u", name="u")
                        if t == 0:
                            # x0 == 0 exactly: grad - G = -G = Aaug[:, :, :, C]
                            nc.vector.tensor_scalar(u[:], aaug_sb[:, :, :, C], -lr,
                                                    None, op0=OP.mult)
                        else:
                            tt = it_pool.tile([128, NBC, C, C + 1], F32, tag="tt",
                                              name="tt")
                            nc.vector.tensor_tensor(
                                tt[:], aaug_sb[:],
                                xaug[:].unsqueeze(2).broadcast_to(
                                    [128, NBC, C, C + 1]), op=OP.mult)
                            grad = it_pool.tile([128, NBC, C], F32, tag="grad",
                                                name="grad")
                            nc.vector.tensor_reduce(grad[:], tt[:], axis=AX.X,
                                                    op=OP.add)
                            nc.vector.scalar_tensor_tensor(u[:], grad[:], -lr,
                                                           xaug[:, :, 0:C],
                                                           op0=OP.mult, op1=OP.add)
                        v = it_pool.tile([128, NBC, C], F32, tag="v", name="v")
                        nc.vector.tensor_scalar(v[:], u[:], 0.0, None, op0=OP.max)
                        s = it_pool.tile([128, NBC, 1], F32, tag="s", name="s")
                        nc.vector.tensor_reduce(s[:], v[:], axis=AX.X, op=OP.add)
                        se = it_pool.tile([128, NBC], F32, tag="se", name="se")
                        nc.vector.tensor_scalar(se[:], s[:].squeeze(2), 1e-8, None,
                                                op0=OP.add)
                        rcp = it_pool.tile([128, NBC], F32, tag="rcp", name="rcp")
                        nc.vector.reciprocal(rcp[:], se[:])
                        nc.vector.tensor_tensor(
                            xaug[:, :, 0:C], v[:],
                            rcp[:].unsqueeze(2).broadcast_to([128, NBC, C]), op=OP.mult)

                    if cycle_ok:
                        xsnap = loop_pool.tile([128, NBC, C], F32)
                        flag_sb = loop_pool.tile([1, 1], F32)
                        ones_sb = loop_pool.tile([128, 1], F32)
                        nc.vector.memset(ones_sb[:], 1.0)
                        for t in range(n_uncond - 2):
                            emit_iter(t)
                        nc.vector.tensor_copy(xsnap[:], xaug[:, :, 0:C])
                        emit_iter(n_uncond - 2)
                        emit_iter(n_uncond - 1)
                        diff = it_pool.tile([128, NBC, C], F32, tag="u", name="diff")
                        nc.vector.tensor_tensor(diff[:], xaug[:, :, 0:C], xsnap[:],
                                                op=OP.subtract)
                        dmax = loop_pool.tile([128, 1], F32)
                        nc.vector.tensor_reduce(dmax[:],
                                                diff[:].rearrange("p c j -> p (c j)"),
                                                axis=AX.X, op=OP.max,
                                                apply_absolute_value=True)
                        with tc.tile_pool(name="flagps", bufs=1, space="PSUM") as fps:
                            fpsum = fps.tile([1, 1], F32)
                            nc.tensor.matmul(fpsum[:], ones_sb[:], dmax[:],
                                             start=True, stop=True)
                            nc.vector.tensor_copy(flag_sb[:], fpsum[:])
                        flag_reg = nc.vector.alloc_register("flag_reg")
                        nc.vector.reg_load(flag_reg,
                                           flag_sb[0:1, 0:1].bitcast(mybir.dt.uint32))
                        rv = nc.vector.snap(flag_reg)
                        if not force_fast_path:
                            with tc.If(rv >= 1):
                                for t in range(n_uncond, n_iters):
                                    emit_iter(t)
                    else:
                        for t in range(n_iters):
                            emit_iter(t)

                # write x out
                nc.sync.dma_start(
                    xout_t.ap().rearrange("(c p) j -> p c j", p=128), xaug[:, :, 0:C])

            # ================= P5: recon = (x @ rp) * cov =================
            if phases >= 5:
                with ExitStack() as ctx:
                    pool = ctx.enter_context(tc.tile_pool(name="p5", bufs=1))
                    io_pool = ctx.enter_context(tc.tile_pool(name="p5io", bufs=4))
                    ps = ctx.enter_context(tc.tile_pool(name="p5ps", bufs=1, space="PSUM"))
                    rp_sb = pool.tile([C, MP], F32R)
                    nc.scalar.dma_start(rp_sb[:], rp_t.ap())
                    xT_sb = pool.tile([C, NBC, 128], F32R)
                    for c in range(NBC):
                        ptx = ps.tile([C, 128], F32, tag="ptx", name="ptx", bufs=2)
                        nc.tensor.transpose(ptx[:], xaug[:, c, 0:C],
                                            ident_sb[:].bitcast(F32))
                        nc.scalar.copy(xT_sb[:, c, :], ptx[:])
                    for c in range(NBC):
                        for ns in range(MT // 4):
                            pr = ps.tile([128, 512], F32, tag="pr", name="pr", bufs=4)
                            nc.tensor.matmul(pr[:], xT_sb[:, c, :],
                                             rp_sb[:, 512 * ns:512 * (ns + 1)],
                                             start=True, stop=True)
                            cvb = io_pool.tile([128, 512], F32, tag="cvb", name="cvb")
                            nc.scalar.dma_start(
                                cvb[:], covB_t.ap()[128 * c:128 * (c + 1),
                                                    512 * ns:512 * (ns + 1)])
                            rt = io_pool.tile([128, 512], F32, tag="rt", name="rt")
                            nc.vector.tensor_tensor(rt[:], pr[:], cvb[:], op=OP.mult)
                            nc.sync.dma_start(
                                recon_t.ap()[128 * c:128 * (c + 1),
                                             512 * ns:512 * (ns + 1)], rt[:])

    nc.compile()
    return nc


_CACHE = {}


def _get_program(step_sizes):
    key = step_sizes.astype(np.float32).tobytes()
    if _CACHE.get("key") != key:
        _CACHE["nc"] = build_program([float(v) for v in step_sizes], len(step_sizes))
        _CACHE["key"] = key
    return _CACHE["nc"]


def _pad_cols(a, width):
    out = np.zeros((a.shape[0], width), dtype=np.float32)
    out[:, :a.shape[1]] = a
    return out


def kernel(marker_props, coverage, W1, b1, W2, b2, step_sizes,
           profile_correction, reference_profiles):
    marker_props = np.asarray(marker_props, dtype=np.float32)
    coverage = np.asarray(coverage, dtype=np.float32)
    W1 = np.asarray(W1, dtype=np.float32)
    b1 = np.asarray(b1, dtype=np.float32)
    W2 = np.asarray(W2, dtype=np.float32)
    b2 = np.asarray(b2, dtype=np.float32)
    step_sizes = np.asarray(step_sizes, dtype=np.float32)
    profile_correction = np.asarray(profile_correction, dtype=np.float32)
    reference_profiles = np.asarray(reference_profiles, dtype=np.float32)

    nc = _get_program(step_sizes)

    # host-side layout prep (pads + transposes)
    w1T = np.zeros((MP, MP), dtype=np.float32)
    w1T[:M, :M] = W1.T
    w2T = np.zeros((MP, MP), dtype=np.float32)
    w2T[:M, :M] = W2.T
    mpT = np.zeros((MP, B), dtype=np.float32)
    mpT[:M] = marker_props.T
    covT = np.zeros((MP, B), dtype=np.float32)
    covT[:M] = coverage.T
    covB = _pad_cols(coverage, MP)
    b1t = np.zeros(MP, dtype=np.float32)
    b1t[:M] = b1
    b1t = np.ascontiguousarray(b1t.reshape(MT, 128).T)
    b2t = np.zeros(MP, dtype=np.float32)
    b2t[:M] = b2
    b2t = np.ascontiguousarray(b2t.reshape(MT, 128).T)
    pc = _pad_cols(profile_correction, MP)
    rp = _pad_cols(reference_profiles, MP)
    ident = np.eye(128, dtype=np.float32)

    in_maps = []
    for c in range(N_CORES):
        sl = slice(BL * c, BL * (c + 1))
        in_maps.append({
            "mpT": np.ascontiguousarray(mpT[:, sl]),
            "covT": np.ascontiguousarray(covT[:, sl]),
            "covB": np.ascontiguousarray(covB[sl]),
            "W1T": w1T, "W2T": w2T,
            "b1t": b1t, "b2t": b2t,
            "pc": pc, "rp": rp, "ident": ident,
        })

    res = run_bass_kernel_spmd(nc, in_maps, core_ids=list(range(N_CORES)))
    x = np.concatenate([res.results[c]["xout"] for c in range(N_CORES)], axis=0)
    recon = np.concatenate([res.results[c]["recon"][:, :M] for c in range(N_CORES)],
                           axis=0)
    return x, np.zeros_like(x), recon


# revision 18
# speedup vs baseline: 1.0205x; 1.0101x over previous
"""Trainium2 Bass kernel for nn_DiffNNLS: 2-layer MLP transform + 100-iter projected-gradient NNLS.

Strategy: pure data-parallel over batch (4096 -> 8 cores x 512). Per core:
  P0: corrected = ref_profiles + sigmoid(profile_correction); corrT via PE transpose
  P1: hT = relu(W1 @ mpT + b1)              (marker-major, fp32r matmuls)
  P2: wT = (W2 @ hT + b2) * covT            (kept in SBUF)
  P3: per-sample Gram matrices A_b = corr diag(cov_b) corr^T via one matmul
      cov^T-contract against pairwise-product columns Q; G = wT^T-contract corrT.
      A and -G packed into augmented tensor Aaug[b, i, j(33)].
  P4: NNLS iterations on DVE: grad-G = reduce_j(Aaug * x_aug); x = normalize(relu(x - lr*(...)))
      With period-2-constant step sizes, x4 == x2 bitwise implies a 2-cycle, so
      iterations 4..N-1 are branch-skipped when the fixed cycle is detected.
  P5: reconstruction = (x @ ref_profiles) * cov
"""
import numpy as np
from contextlib import ExitStack

import concourse.bass as bass
import concourse.tile as tile
from concourse import bacc, mybir
from concourse.bass_utils import run_bass_kernel_spmd

F32 = mybir.dt.float32
F32R = mybir.dt.float32r
AX = mybir.AxisListType
OP = mybir.AluOpType
ACTF = mybir.ActivationFunctionType

B, M, C = 4096, 5000, 32
N_CORES = 8
BL = B // N_CORES            # 512 samples per core
NBC = BL // 128              # 4 batch chunks of 128
MP = 5120                    # padded marker dim (40 * 128)
MT = MP // 128               # 40 marker tiles
KSLABS = MP // 1024          # 5 slabs of 1024 output features


def build_program(lrs, n_iters, phases=6, force_fast_path=False):
    """Emit the SPMD program. lrs: list of per-iteration step sizes (floats)."""
    nc = bacc.Bacc("TRN2", target_bir_lowering=False, debug=False, num_devices=N_CORES)

    mpT_t = nc.dram_tensor("mpT", [MP, BL], F32R, kind="ExternalInput")
    covT_t = nc.dram_tensor("covT", [MP, BL], F32R, kind="ExternalInput")
    covB_t = nc.dram_tensor("covB", [BL, MP], F32, kind="ExternalInput")
    w1T_t = nc.dram_tensor("W1T", [MP, MP], F32R, kind="ExternalInput")
    w2T_t = nc.dram_tensor("W2T", [MP, MP], F32R, kind="ExternalInput")
    b1t_t = nc.dram_tensor("b1t", [128, MT], F32, kind="ExternalInput")
    b2t_t = nc.dram_tensor("b2t", [128, MT], F32, kind="ExternalInput")
    pc_t = nc.dram_tensor("pc", [C, MP], F32, kind="ExternalInput")
    rp_t = nc.dram_tensor("rp", [C, MP], F32R, kind="ExternalInput")
    id_t = nc.dram_tensor("ident", [128, 128], F32R, kind="ExternalInput")
    xout_t = nc.dram_tensor("xout", [BL, C], F32, kind="ExternalOutput")
    recon_t = nc.dram_tensor("recon", [BL, MP], F32, kind="ExternalOutput")

    # early-exit is sound when the iteration map repeats with period 2 from iter 2 on
    n_uncond = 4 if n_iters % 2 == 0 else 5
    cycle_ok = n_iters > n_uncond and all(
        lrs[t] == lrs[t - 2] for t in range(n_uncond - 2, n_iters))

    with tile.TileContext(nc) as tc:
        with ExitStack() as top:
            # ---- long-lived tiles ----
            persist = top.enter_context(tc.tile_pool(name="persist", bufs=1))
            corrT_sb = persist.tile([128, MT, C], F32R)       # 5.1KB/part
            b1t_sb = persist.tile([128, MT], F32)
            b2t_sb = persist.tile([128, MT], F32)
            ident_sb = persist.tile([128, 128], F32R)
            nc.scalar.dma_start(b1t_sb[:], b1t_t.ap())
            nc.scalar.dma_start(b2t_sb[:], b2t_t.ap())
            nc.scalar.dma_start(ident_sb[:], id_t.ap())

            aaug_sb = top.enter_context(tc.tile_pool(name="aaug", bufs=1)).tile(
                [128, NBC, C, C + 1], F32, name="aaug_sb")
            loop_pool = top.enter_context(tc.tile_pool(name="loop", bufs=1))
            xaug = loop_pool.tile([128, NBC, C + 1], F32, name="xaug")

            if phases < 0:
                stub = loop_pool.tile([128, C], F32, name="stub")
                nc.vector.memset(stub[:], 0.0)
                nc.sync.dma_start(xout_t.ap().rearrange("(c p) j -> p c j", p=128)[:, 0, :],
                                  stub[:])

            # ================= P0: corrected profiles, transposed =================
            if phases >= 0:
                with ExitStack() as ctx:
                    pool = ctx.enter_context(tc.tile_pool(name="p0", bufs=1))
                    ps = ctx.enter_context(tc.tile_pool(name="p0ps", bufs=4, space="PSUM"))
                    pc_sb = pool.tile([C, MP], F32)
                    nc.scalar.dma_start(pc_sb[:], pc_t.ap())
                    rp_sb0 = pool.tile([C, MP], F32R)
                    nc.scalar.dma_start(rp_sb0[:], rp_t.ap())
                    sig_sb = pool.tile([C, MP], F32)
                    nc.scalar.activation(sig_sb[:], pc_sb[:], ACTF.Sigmoid)
                    corr_sb = pool.tile([C, MP], F32R)
                    nc.vector.tensor_tensor(corr_sb[:], rp_sb0[:].bitcast(F32),
                                            sig_sb[:], op=OP.add)
                    for mt in range(MT):
                        ptr = ps.tile([128, C], F32R, tag="ptr", name="ptr")
                        nc.tensor.transpose(ptr[:], corr_sb[:, 128 * mt:128 * (mt + 1)],
                                            ident_sb[0:C, 0:C])
                        nc.scalar.copy(corrT_sb[:, mt, :], ptr[:])

            # ================= P1: hT = relu(W1 @ mpT + b1) =================
            big_stack = ExitStack()
            big_pool = big_stack.enter_context(tc.tile_pool(name="big", bufs=2))
            mpT_sb = big_pool.tile([128, MT, BL], F32R, tag="big", name="mpT_sb")
            hT_sb = big_pool.tile([128, MT, BL], F32R, tag="big", name="hT_sb")
            if phases >= 1:
                with ExitStack() as ctx:
                    w_pool = ctx.enter_context(tc.tile_pool(name="w1s", bufs=8))
                    ps = ctx.enter_context(tc.tile_pool(name="p1ps", bufs=8, space="PSUM"))
                    nc.scalar.dma_start(mpT_sb[:],
                                        mpT_t.ap().rearrange("(t p) c -> p t c", p=128))
                    for s in range(2 * KSLABS):
                        accs = [ps.tile([128, BL], F32, tag="acc", name="acc")
                                for _ in range(4)]
                        for mt in range(MT):
                            w1t = w_pool.tile([128, 512], F32R, tag="w", name="w1t")
                            nc.sync.dma_start(
                                w1t[:], w1T_t.ap()[128 * mt:128 * (mt + 1),
                                                   512 * s:512 * (s + 1)])
                            for jj in range(4):
                                nc.tensor.matmul(accs[jj][:],
                                                 w1t[:, 128 * jj:128 * (jj + 1)],
                                                 mpT_sb[:, mt, :],
                                                 start=(mt == 0), stop=(mt == MT - 1))
                        for jj in range(4):
                            kt = 4 * s + jj
                            if jj < 2:
                                nc.scalar.activation(hT_sb[:, kt, :], accs[jj][:],
                                                     ACTF.Relu,
                                                     bias=b1t_sb[:, kt:kt + 1])
                            else:
                                nc.vector.tensor_scalar(hT_sb[:, kt, :], accs[jj][:],
                                                        b1t_sb[:, kt:kt + 1], 0.0,
                                                        op0=OP.add, op1=OP.max)

            # ================= P2: wT = (W2 @ hT + b2) * covT (SBUF-resident) ========
            wT_sb = big_pool.tile([128, MT, BL], F32R, tag="big", name="wT_sb")
            if phases >= 2:
                with ExitStack() as ctx:
                    w_pool = ctx.enter_context(tc.tile_pool(name="w2s", bufs=4))
                    cv_pool = ctx.enter_context(tc.tile_pool(name="cv", bufs=3))
                    ps = ctx.enter_context(tc.tile_pool(name="p2ps", bufs=8, space="PSUM"))
                    for s in range(2 * KSLABS):
                        accs = [ps.tile([128, BL], F32, tag="acc", name="acc")
                                for _ in range(4)]
                        for kt in range(MT):
                            w2t = w_pool.tile([128, 512], F32R, tag="w", name="w2t")
                            nc.sync.dma_start(
                                w2t[:], w2T_t.ap()[128 * kt:128 * (kt + 1),
                                                   512 * s:512 * (s + 1)])
                            for jj in range(4):
                                nc.tensor.matmul(accs[jj][:],
                                                 w2t[:, 128 * jj:128 * (jj + 1)],
                                                 hT_sb[:, kt, :],
                                                 start=(kt == 0), stop=(kt == MT - 1))
                        for jj in range(4):
                            mt2 = 4 * s + jj
                            cvt = cv_pool.tile([128, BL], F32R, tag="cv", name="cvt")
                            nc.scalar.dma_start(cvt[:],
                                                covT_t.ap()[128 * mt2:128 * (mt2 + 1), :])
                            nc.vector.scalar_tensor_tensor(
                                wT_sb[:, mt2, :], accs[jj][:], b2t_sb[:, mt2:mt2 + 1],
                                cvt[:].bitcast(F32), op0=OP.add, op1=OP.mult)

            # ================= P3: Aaug = [A | -G] =================
            if phases >= 3:
                # G pass: G[b,i] = sum_m wT[m,b] corrT[m,i]
                with ExitStack() as ctx:
                    ps = ctx.enter_context(tc.tile_pool(name="p3g", bufs=1, space="PSUM"))
                    pG = [ps.tile([128, C], F32, tag=f"pg{c}", name=f"pg{c}")
                          for c in range(NBC)]
                    for mt in range(MT):
                        st, sp = (mt == 0), (mt == MT - 1)
                        for c in range(NBC):
                            nc.tensor.matmul(pG[c][:],
                                             wT_sb[:, mt, 128 * c:128 * (c + 1)],
                                             corrT_sb[:, mt, :], start=st, stop=sp)
                    for c in range(NBC):
                        nc.vector.tensor_scalar(aaug_sb[:, c, :, C], pG[c][:], -1.0,
                                                None, op0=OP.mult)
                # A pass: Avec[b,(i,j)] = sum_m covT[m,b] corr[i,m] corr[j,m]
                with ExitStack() as ctx:
                    cv_pool = ctx.enter_context(tc.tile_pool(name="cv3", bufs=3))
                    q_pool = ctx.enter_context(tc.tile_pool(name="q3", bufs=3))
                    ps = ctx.enter_context(tc.tile_pool(name="p3a", bufs=1, space="PSUM"))
                    pA = [[ps.tile([128, 512], F32, tag=f"pa{c}{h}", name=f"pa{c}{h}")
                           for h in range(2)] for c in range(NBC)]
                    for mt in range(MT):
                        cvt = cv_pool.tile([128, BL], F32R, tag="cv", name="cvt3")
                        nc.scalar.dma_start(cvt[:],
                                            covT_t.ap()[128 * mt:128 * (mt + 1), :])
                        qt = q_pool.tile([128, C * C], F32R, tag="q", name="qt")
                        ct = corrT_sb[:, mt, :]
                        nc.vector.tensor_tensor(
                            qt[:].rearrange("p (i j) -> p i j", i=C),
                            ct.unsqueeze(2).broadcast_to([128, C, C]),
                            ct.unsqueeze(1).broadcast_to([128, C, C]), op=OP.mult)
                        st, sp = (mt == 0), (mt == MT - 1)
                        for c in range(NBC):
                            lhs_cv = cvt[:, 128 * c:128 * (c + 1)]
                            nc.tensor.matmul(pA[c][0][:], lhs_cv, qt[:, 0:512],
                                             start=st, stop=sp)
                            nc.tensor.matmul(pA[c][1][:], lhs_cv, qt[:, 512:1024],
                                             start=st, stop=sp)
                    for c in range(NBC):
                        for h in range(2):
                            nc.vector.tensor_copy(
                                aaug_sb[:, c, 16 * h:16 * (h + 1), 0:C],
                                pA[c][h][:].rearrange("p (i j) -> p i j", i=16))

            big_stack.close()

            # ================= P4: the NNLS loop =================
            if phases >= 4:
                nc.vector.memset(xaug[:, :, 0:C], 0.0)
                nc.vector.memset(xaug[:, :, C:C + 1], 1.0)
                with ExitStack() as ctx:
                    it_pool = ctx.enter_context(tc.tile_pool(name="it", bufs=1))

                    def emit_iter(t):
                        lr = float(lrs[t])
                        u = it_pool.tile([128, NBC, C], F32, tag="u", name="u")
                        if t == 0:
                            # x0 == 0 exactly: grad - G = -G = Aaug[:, :, :, C]
                            nc.vector.tensor_scalar(u[:], aaug_sb[:, :, :, C], -lr,
                                                    None, op0=OP.mult)
                        else:
                            tt = it_pool.tile([128, NBC, C, C + 1], F32, tag="tt",
                                              name="tt")
                            nc.vector.tensor_tensor(
                                tt[:], aaug_sb[:],
                                xaug[:].unsqueeze(2).broadcast_to(
                                    [128, NBC, C, C + 1]), op=OP.mult)
                            grad = it_pool.tile([128, NBC, C], F32, tag="grad",
                                                name="grad")
                            nc.vector.tensor_reduce(grad[:], tt[:], axis=AX.X,
                                                    op=OP.add)
                            nc.vector.scalar_tensor_tensor(u[:], grad[:], -lr,
                                                           xaug[:, :, 0:C],
                                                           op0=OP.mult, op1=OP.add)
                        v = it_pool.tile([128, NBC, C], F32, tag="v", name="v")
                        nc.vector.tensor_scalar(v[:], u[:], 0.0, None, op0=OP.max)
                        s = it_pool.tile([128, NBC, 1], F32, tag="s", name="s")
                        nc.vector.tensor_reduce(s[:], v[:], axis=AX.X, op=OP.add)
                        se = it_pool.tile([128, NBC], F32, tag="se", name="se")
                        nc.vector.tensor_scalar(se[:], s[:].squeeze(2), 1e-8, None,
                                                op0=OP.add)
                        rcp = it_pool.tile([128, NBC], F32, tag="rcp", name="rcp")
                        nc.vector.reciprocal(rcp[:], se[:])
                        nc.vector.tensor_tensor(
                            xaug[:, :, 0:C], v[:],
                            rcp[:].unsqueeze(2).broadcast_to([128, NBC, C]), op=OP.mult)

                    if cycle_ok:
                        xsnap = loop_pool.tile([128, NBC, C], F32)
                        flag_sb = loop_pool.tile([1, 1], F32)
                        ones_sb = loop_pool.tile([128, 1], F32)
                        nc.vector.memset(ones_sb[:], 1.0)
                        for t in range(n_uncond - 2):
                            emit_iter(t)
                        nc.vector.tensor_copy(xsnap[:], xaug[:, :, 0:C])
                        emit_iter(n_uncond - 2)
                        emit_iter(n_uncond - 1)
                        diff = it_pool.tile([128, NBC, C], F32, tag="u", name="diff")
                        nc.vector.tensor_tensor(diff[:], xaug[:, :, 0:C], xsnap[:],
                                                op=OP.subtract)
                        dmax = loop_pool.tile([128, 1], F32)
                        nc.vector.tensor_reduce(dmax[:],
                                                diff[:].rearrange("p c j -> p (c j)"),
                                                axis=AX.X, op=OP.max,
                                                apply_absolute_value=True)
                        with tc.tile_pool(name="flagps", bufs=1, space="PSUM") as fps:
                            fpsum = fps.tile([1, 1], F32)
                            nc.tensor.matmul(fpsum[:], ones_sb[:], dmax[:],
                                             start=True, stop=True)
                            nc.vector.tensor_copy(flag_sb[:], fpsum[:])
                        flag_reg = nc.vector.alloc_register("flag_reg")
                        nc.vector.reg_load(flag_reg,
                                           flag_sb[0:1, 0:1].bitcast(mybir.dt.uint32))
                        rv = nc.vector.snap(flag_reg)
                        if not force_fast_path:
                            with tc.If(rv >= 1):
                                for t in range(n_uncond, n_iters):
                                    emit_iter(t)
                    else:
                        for t in range(n_iters):
                            emit_iter(t)

                # write x out
                nc.sync.dma_start(
                    xout_t.ap().rearrange("(c p) j -> p c j", p=128), xaug[:, :, 0:C])

            # ================= P5: recon = (x @ rp) * cov =================
            if phases >= 5:
                with ExitStack() as ctx:
                    pool = ctx.enter_context(tc.tile_pool(name="p5", bufs=1))
                    io_pool = ctx.enter_context(tc.tile_pool(name="p5io", bufs=4))
                    ps = ctx.enter_context(tc.tile_pool(name="p5ps", bufs=1, space="PSUM"))
                    rp_sb = pool.tile([C, MP], F32R)
                    nc.scalar.dma_start(rp_sb[:], rp_t.ap())
                    xT_sb = pool.tile([C, NBC, 128], F32R)
                    for c in range(NBC):
                        ptx = ps.tile([C, 128], F32, tag="ptx", name="ptx", bufs=2)
                        nc.tensor.transpose(ptx[:], xaug[:, c, 0:C],
                                            ident_sb[:].bitcast(F32))
                        nc.scalar.copy(xT_sb[:, c, :], ptx[:])
                    for c in range(NBC):
                        for ns in range(MT // 4):
                            pr = ps.tile([128, 512], F32, tag="pr", name="pr", bufs=4)
                            nc.tensor.matmul(pr[:], xT_sb[:, c, :],
                                             rp_sb[:, 512 * ns:512 * (ns + 1)],
                                             start=True, stop=True)
                            cvb = io_pool.tile([128, 512], F32, tag="cvb", name="cvb")
                            nc.scalar.dma_start(
                                cvb[:], covB_t.ap()[128 * c:128 * (c + 1),
                                                    512 * ns:512 * (ns + 1)])
                            rt = io_pool.tile([128, 512], F32, tag="rt", name="rt")
                            nc.vector.tensor_tensor(rt[:], pr[:], cvb[:], op=OP.mult)
                            nc.sync.dma_start(
                                recon_t.ap()[128 * c:128 * (c + 1),
                                             512 * ns:512 * (ns + 1)], rt[:])

    nc.compile()
    return nc


_CACHE = {}


def _get_program(step_sizes):
    key = step_sizes.astype(np.float32).tobytes()
    if _CACHE.get("key") != key:
        _CACHE["nc"] = build_program([float(v) for v in step_sizes], len(step_sizes))
        _CACHE["key"] = key
    return _CACHE["nc"]


def _pad_cols(a, width):
    out = np.zeros((a.shape[0], width), dtype=np.float32)
    out[:, :a.shape[1]] = a
    return out


def kernel(marker_props, coverage, W1, b1, W2, b2, step_sizes,
           profile_correction, reference_profiles):
    marker_props = np.asarray(marker_props, dtype=np.float32)
    coverage = np.asarray(coverage, dtype=np.float32)
    W1 = np.asarray(W1, dtype=np.float32)
    b1 = np.asarray(b1, dtype=np.float32)
    W2 = np.asarray(W2, dtype=np.float32)
    b2 = np.asarray(b2, dtype=np.float32)
    step_sizes = np.asarray(step_sizes, dtype=np.float32)
    profile_correction = np.asarray(profile_correction, dtype=np.float32)
    reference_profiles = np.asarray(reference_profiles, dtype=np.float32)

    nc = _get_program(step_sizes)

    # host-side layout prep (pads + transposes)
    w1T = np.zeros((MP, MP), dtype=np.float32)
    w1T[:M, :M] = W1.T
    w2T = np.zeros((MP, MP), dtype=np.float32)
    w2T[:M, :M] = W2.T
    mpT = np.zeros((MP, B), dtype=np.float32)
    mpT[:M] = marker_props.T
    covT = np.zeros((MP, B), dtype=np.float32)
    covT[:M] = coverage.T
    covB = _pad_cols(coverage, MP)
    b1t = np.zeros(MP, dtype=np.float32)
    b1t[:M] = b1
    b1t = np.ascontiguousarray(b1t.reshape(MT, 128).T)
    b2t = np.zeros(MP, dtype=np.float32)
    b2t[:M] = b2
    b2t = np.ascontiguousarray(b2t.reshape(MT, 128).T)
    pc = _pad_cols(profile_correction, MP)
    rp = _pad_cols(reference_profiles, MP)
    ident = np.eye(128, dtype=np.float32)

    in_maps = []
    for c in range(N_CORES):
        sl = slice(BL * c, BL * (c + 1))
        in_maps.append({
            "mpT": np.ascontiguousarray(mpT[:, sl]),
            "covT": np.ascontiguousarray(covT[:, sl]),
            "covB": np.ascontiguousarray(covB[sl]),
            "W1T": w1T, "W2T": w2T,
            "b1t": b1t, "b2t": b2t,
            "pc": pc, "rp": rp, "ident": ident,
        })

    res = run_bass_kernel_spmd(nc, in_maps, core_ids=list(range(N_CORES)))
    x = np.concatenate([res.results[c]["xout"] for c in range(N_CORES)], axis=0)
    recon = np.concatenate([res.results[c]["recon"][:, :M] for c in range(N_CORES)],
                           axis=0)
    return x, np.zeros_like(x), recon
